# revision 16
# baseline (speedup 1.0000x reference)
"""Trainium2 Bass kernel for DYSPN-style dynamic local filtering (fp16).

Computation (per batch b, pixel p):
    patches[j,p] = 7x7 im2col of `input` (zero pad 3), center tap replaced by input0
    scale[j,p]   = attention[b, i, ring(j), p]      (ring in {0..3}, scale >= 0)
    w            = kernel * scale;  w /= sum_j |w|
    out[p]       = sum_j patches[j,p] * w[j,p]

Since scale >= 0 and constant within a ring (ring = Chebyshev distance from
the center tap):
    out = (sum_r att_r * B_r) / (sum_r att_r * A_r)
    B_r = sum_{j in ring r} patches_j * k_j,   A_r = sum_{j in ring r} |k_j|

Sharding: 8 cores = 4 batches x 2 half-images (128 rows each). Per core the
output plane is [128 rows (partitions), 320 cols (free)]; tap shifts become
free-dim offsets into 7 pre-shifted padded-image variants (host-built).

All tensors are fp16 on device: DVE tensor_tensor hits the 2X_1PORT mode
(2 elems/cycle, measured 0.52 ns/elem vs 1.04 fp32) and DMA bytes halve.
Measured on HW: odd element offsets and stride-0 broadcast dims keep the 2x
mode; GpSimd is never used (a concurrent POOL op slows DVE ~13x). |k| runs
on ScalarE (own SBUF port, ~0.9 ns/elem). Products land in a separate tile
from k (no WAR hazard against ScalarE's |k| reads). The |k| planes live 49
planes above the product planes in one tile so each B-tree op also carries
the matching A-tree level as a second AP dim. Final division is fp32 via
reciprocal_approx_accurate. Tolerance is 2e-2; fp16 pipeline sims at 8e-4.
"""

import sys

for _p in ("/opt/trn_rl_repo", "/root/.axon_site"):
    if _p not in sys.path:
        sys.path.insert(0, _p)

import types
import numpy as np
from contextlib import ExitStack

import concourse.bass as bass
import concourse.tile as tile
from concourse import bacc, mybir
from concourse.bass_utils import run_bass_kernel_spmd
from concourse.vector_clock import ScopedClock


def _lean_epilogue(self, tick_clock, wait_clock):
    """Replaces TileContext._drain_and_barrier for this kernel.

    The stock epilogue costs ~10us on HW: drain + full all-engine barrier
    (~3us of semaphore-propagation latency), per-semaphore clears, then a
    second all-engine barrier. Here GpSimd alone waits for every engine's
    completion clock (same wait set the stock drain used, so the out-DMA
    completion is included), resets DMA state and clears the semaphores for
    re-run correctness; no global barriers. NEFF completion still requires
    all sequencers idle, which orders run N's clears before run N+1.
    """
    nc = self.nc
    drain_inst = nc.gpsimd.drain()
    wait_clock.add_sem_waits(
        drain_inst.ins, ScopedClock({None: tick_clock.global_clock}))
    popped = nc._tile_sem_poison_stack.pop()
    assert popped is self._sem_poison
    nc.clear_and_free_semaphores(list(self.sems.allocated().values()))

H, W = 256, 320
BS = 4
KK = 49
HALF_ROWS = 128
PAD_W = W + 6  # 326
APLANE = 49  # |k| plane j lives at kall plane j + 49

def _ring_ids() -> np.ndarray:
    ring = np.zeros(KK, dtype=np.int32)
    for j in range(KK):
        dy, dx = divmod(j, 7)
        ring[j] = max(abs(dy - 3), abs(dx - 3))
    return ring

_RING = _ring_ids()
RING_TAPS = [np.where(_RING == r)[0].tolist() for r in range(4)]  # 1,8,16,24
RING_ORDER = np.concatenate([np.asarray(t) for t in RING_TAPS]).astype(np.int64)

# plane ranges of each ring inside the [128, 49, 320] ring-ordered k region
RING_OFF = [0, 1, 9, 25, 49]

def _mul_ops(r):
    """Tap-multiply op shapes for ring r>=1: (rel_plane, n_planes, img_dims, img_off).

    Ring taps in j-order: top row (2r+1), middle 2r-1 rows with dx in
    {3-r, 3+r}, bottom row (2r+1). img_dims are AP dims [stride, num]
    prepended to [1, W]; img_off indexes the [7, 326] shifted-image block.
    """
    n = 2 * r + 1
    lo = 3 - r
    return [
        (0, n, [[1, n]], lo * PAD_W + lo),
        (n, 2 * (n - 2), [[PAD_W, n - 2], [2 * r, 2]], (lo + 1) * PAD_W + lo),
        (n + 2 * (n - 2), n, [[1, n]], (lo + n - 1) * PAD_W + lo),
    ]

_NC = None
LAST_RESULTS = None


def _build_program():
    f16 = mybir.dt.float16
    f32 = mybir.dt.float32
    nc = bacc.Bacc("TRN2", target_bir_lowering=False, debug=False, num_devices=8)
    k_d = nc.dram_tensor("k", [HALF_ROWS, KK, W], f16, kind="ExternalInput").ap()
    img7_d = nc.dram_tensor("img7", [HALF_ROWS, 7, PAD_W], f16, kind="ExternalInput").ap()
    in0_d = nc.dram_tensor("in0", [HALF_ROWS, W], f16, kind="ExternalInput").ap()
    att_d = nc.dram_tensor("att", [HALF_ROWS, 4, W], f16, kind="ExternalInput").ap()
    out_d = nc.dram_tensor("out", [HALF_ROWS, W], f16, kind="ExternalOutput").ap()

    tc_obj = tile.TileContext(nc)
    tc_obj._drain_and_barrier = types.MethodType(_lean_epilogue, tc_obj)
    with tc_obj as tc, ExitStack() as ctx:
        pool = ctx.enter_context(tc.tile_pool(name="main", bufs=1))

        k16 = pool.tile([HALF_ROWS, KK, W], f16, name="k16")        # raw k
        # planes 0:49 = patches*k (ring-ordered), planes 49:98 = |k|
        kall = pool.tile([HALF_ROWS, 98, W], f16, name="kall")
        img7_t = pool.tile([HALF_ROWS, 7, PAD_W], f16)
        in0_t = pool.tile([HALF_ROWS, W], f16)
        att_t = pool.tile([HALF_ROWS, 4, W], f16)
        # planes 0:4 = B_r, planes 4:8 = A_r
        res = pool.tile([HALF_ROWS, 8, W], f16)
        pnd = pool.tile([HALF_ROWS, 8, W], f16)

        kall_ap = kall[:]
        kpart = kall_ap.ap[0]
        img7_ap = img7_t[:]
        ipart = img7_ap.ap[0]
        att_ap = att_t[:]

        def kap(plane, dims):
            return bass.AP(kall_ap.tensor, kall_ap.offset + plane * W,
                           [kpart] + dims)

        def iap(off, dims):
            return bass.AP(img7_ap.tensor, img7_ap.offset + off,
                           [ipart] + dims + [[1, W]])

        # ---- DMAs, ordered by when compute needs the data; the first k
        # chunks lead so ScalarE/DVE start ASAP. All transfers cover exact
        # contiguous plane ranges (a strided multi-plane transfer makes the
        # dependency tracker alias its whole bounding box, serializing
        # unrelated compute against the last image rows).
        # k chunks flow from the Sync HWDGE ring; the small early image/in0
        # transfers issue in parallel from the Scalar HWDGE ring so their
        # descriptors don't serialize behind k's.
        nc.sync.dma_start(k16[:, 0:5, :], k_d[:, 0:5, :])           # center+r1 top
        nc.scalar.dma_start(img7_t[:, 2:5, :], img7_d[:, 2:5, :])   # rows 2,3,4
        nc.scalar.dma_start(in0_t[:], in0_d[:])
        nc.sync.dma_start(k16[:, 5:9, :], k_d[:, 5:9, :])           # ring1 rest
        nc.sync.dma_start(k16[:, 9:25, :], k_d[:, 9:25, :])         # ring2
        nc.scalar.dma_start(att_t[:], att_d[:])
        nc.sync.dma_start(img7_t[:, 1:2, :], img7_d[:, 1:2, :])
        nc.sync.dma_start(img7_t[:, 5:6, :], img7_d[:, 5:6, :])
        nc.sync.dma_start(k16[:, 25:33, :], k_d[:, 25:33, :])       # ring3
        nc.sync.dma_start(img7_t[:, 0:1, :], img7_d[:, 0:1, :])
        nc.sync.dma_start(img7_t[:, 6:7, :], img7_d[:, 6:7, :])
        nc.sync.dma_start(k16[:, 33:41, :], k_d[:, 33:41, :])
        nc.sync.dma_start(k16[:, 41:49, :], k_d[:, 41:49, :])

        # ---- |k| on ScalarE (reads k16, writes kall[49:98] / res[4]),
        # chunked to follow the k DMA arrivals
        Abs = mybir.ActivationFunctionType.Abs
        nc.scalar.activation(kall[:, 50:54, :], k16[:, 1:5, :], Abs)  # r1 top
        nc.scalar.activation(res[:, 4, :], k16[:, 0, :], Abs)         # A_0
        nc.scalar.activation(kall[:, 54:58, :], k16[:, 5:9, :], Abs)  # r1 rest
        nc.scalar.activation(kall[:, 58:74, :], k16[:, 9:25, :], Abs) # ring2
        nc.scalar.activation(kall[:, 74:90, :], k16[:, 25:41, :], Abs)# ring3a
        nc.scalar.activation(kall[:, 90:98, :], k16[:, 41:49, :], Abs)# ring3b

        # ---- ring sums: each op handles the B level and the A level (49
        # planes up) through a paired leading AP dim
        def paired_fold(base, h, delta):
            """kall[{base, base+49}][0:h] += kall[{base+delta, ...}][0:h]"""
            dims = [[APLANE * W, 2], [W, h], [1, W]]
            nc.vector.tensor_add(kap(base, dims), kap(base, dims),
                                 kap(base + delta, dims))

        def paired_tree(base, sz, r):
            cur = sz
            while cur > 2:
                paired_fold(base, cur // 2, cur // 2)
                cur //= 2
            dims = [[APLANE * W, 2], [1, W]]
            rdims = [[4 * W, 2], [1, W]]
            nc.vector.tensor_add(
                bass.AP(res[:].tensor, res[:].offset + r * W, [res[:].ap[0]] + rdims),
                kap(base, dims), kap(base + 1, dims))

        def ring_muls(r):
            for (rel, n_pl, img_dims, img_off) in _mul_ops(r):
                o = RING_OFF[r] + rel
                nc.vector.tensor_mul(kall[:, o:o + n_pl, :],
                                     k16[:, o:o + n_pl, :],
                                     iap(img_off, img_dims))

        pnd_ap = pnd[:]
        ppart = pnd_ap.ap[0]
        d1 = [[4 * W, 2], [1, W]]

        def pap(plane, dims):
            return bass.AP(pnd_ap.tensor, pnd_ap.offset + plane * W,
                           [ppart] + dims)

        def pnd_mul(r, n):
            """pnd{r..r+n, 4+r..} = res{...} * att{r..r+n} (B|A paired)."""
            dims = [[4 * W, 2], [W, n], [1, W]] if n > 1 else d1
            att_b = bass.AP(att_ap.tensor, att_ap.offset + r * W,
                            [att_ap.ap[0]] + ([[0, 2], [W, n], [1, W]]
                                              if n > 1 else [[0, 2], [1, W]]))
            rsrc = bass.AP(res[:].tensor, res[:].offset + r * W,
                           [res[:].ap[0]] + dims)
            nc.vector.tensor_mul(pap(r, dims), rsrc, att_b)

        # ---- DVE stream, ordered by operand readiness (in-order engine:
        # an op that waits blocks everything issued after it)
        ring_muls(1)
        nc.vector.tensor_mul(res[:, 0, :], k16[:, 0, :], in0_t[:])     # B_0
        paired_tree(1, 8, 1)                     # ring1
        pnd_mul(0, 2)                            # pnd{0,1,4,5} = res*att
        nc.vector.tensor_add(pap(0, d1), pap(0, d1), pap(1, d1))
        ring_muls(2)
        ring_muls(3)
        paired_tree(9, 16, 2)                    # ring2
        pnd_mul(2, 1)
        paired_fold(25, 8, 8)                    # ring3: fold chunks
        paired_fold(25, 8, 16)
        paired_tree(25, 8, 3)
        pnd_mul(3, 1)
        nc.vector.tensor_add(pap(2, d1), pap(2, d1), pap(3, d1))

        nd32 = pool.tile([HALF_ROWS, 2, W], f32)
        nd_dims = [[W, 2], [1, W]]
        nd_ap = bass.AP(nd32[:].tensor, nd32[:].offset, [nd32[:].ap[0]] + nd_dims)
        nc.vector.tensor_add(nd_ap, pap(0, d1), pap(2, d1))  # fp32 N, D

        rden_t = pool.tile([HALF_ROWS, W], f32)
        scr_t = pool.tile([HALF_ROWS, W], f32)
        nc.vector.reciprocal_approx_accurate(rden_t[:], nd32[:, 1, :], scr_t[:])
        out_t = pool.tile([HALF_ROWS, W], f16)
        nc.vector.tensor_mul(out_t[:], nd32[:, 0, :], rden_t[:])
        nc.sync.dma_start(out_d[:], out_t[:])

    nc.compile()
    return nc


def _get_program():
    global _NC
    if _NC is None:
        _NC = _build_program()
    return _NC


def kernel(**inputs) -> np.ndarray:
    k = np.asarray(inputs["kernel"], dtype=np.float32)      # [4, 49, 81920]
    img = np.asarray(inputs["input"], dtype=np.float32)     # [4, 1, 256, 320]
    in0 = np.asarray(inputs["input0"], dtype=np.float32)    # [4, 1, 256, 320]
    att = np.asarray(inputs["attention"], dtype=np.float32) # [4, 6, 4, 81920]
    ii = int(np.asarray(inputs["i"]))

    nc = _get_program()

    in_maps = []
    for c in range(8):
        b, half = divmod(c, 2)
        y0 = half * HALF_ROWS
        kc = k[b][RING_ORDER][:, y0 * W:(y0 + HALF_ROWS) * W]
        kc = np.ascontiguousarray(
            kc.reshape(KK, HALF_ROWS, W).transpose(1, 0, 2).astype(np.float16))
        pad = np.zeros((HALF_ROWS + 6, PAD_W), np.float16)
        lo, hi = max(0, y0 - 3), min(H, y0 + HALF_ROWS + 3)
        pad[lo - (y0 - 3):hi - (y0 - 3), 3:3 + W] = img[b, 0, lo:hi]
        img7 = np.ascontiguousarray(
            np.stack([pad[t:t + HALF_ROWS] for t in range(7)], axis=1))
        in0c = np.ascontiguousarray(in0[b, 0, y0:y0 + HALF_ROWS]).astype(np.float16)
        attc = att[b, ii][:, y0 * W:(y0 + HALF_ROWS) * W]
        attc = np.ascontiguousarray(
            attc.reshape(4, HALF_ROWS, W).transpose(1, 0, 2).astype(np.float16))
        in_maps.append({"k": kc, "img7": img7, "in0": in0c, "att": attc})

    res = run_bass_kernel_spmd(nc, in_maps, list(range(8)))
    global LAST_RESULTS
    LAST_RESULTS = res

    out = np.empty((BS, 1, H, W), np.float32)
    for c in range(8):
        b, half = divmod(c, 2)
        out[b, 0, half * HALF_ROWS:(half + 1) * HALF_ROWS] = \
            res.results[c]["out"].astype(np.float32)
    return out


# revision 18
# speedup vs baseline: 1.0681x; 1.0681x over previous
"""Trainium2 Bass kernel for DYSPN-style dynamic local filtering (fp16).

Computation (per batch b, pixel p):
    patches[j,p] = 7x7 im2col of `input` (zero pad 3), center tap replaced by input0
    scale[j,p]   = attention[b, i, ring(j), p]      (ring in {0..3}, scale >= 0)
    w            = kernel * scale;  w /= sum_j |w|
    out[p]       = sum_j patches[j,p] * w[j,p]

Since scale >= 0 and constant within a ring (ring = Chebyshev distance from
the center tap):
    out = (sum_r att_r * B_r) / (sum_r att_r * A_r)
    B_r = sum_{j in ring r} patches_j * k_j,   A_r = sum_{j in ring r} |k_j|

Sharding: 8 cores = 4 batches x 2 half-images (128 rows each). Per core the
output plane is [128 rows (partitions), 320 cols (free)]; tap shifts become
free-dim offsets into 7 pre-shifted padded-image variants (host-built).

All tensors are fp16 on device: DVE tensor_tensor hits the 2X_1PORT mode
(2 elems/cycle, measured 0.52 ns/elem vs 1.04 fp32) and DMA bytes halve.
Measured on HW: odd element offsets and stride-0 broadcast dims keep the 2x
mode; GpSimd is never used (a concurrent POOL op slows DVE ~13x). |k| runs
on ScalarE (own SBUF port, ~0.9 ns/elem). Products land in a separate tile
from k (no WAR hazard against ScalarE's |k| reads). The |k| planes live 49
planes above the product planes in one tile so each B-tree op also carries
the matching A-tree level as a second AP dim. Final division is fp32 via
reciprocal_approx_accurate. Tolerance is 2e-2; fp16 pipeline sims at 8e-4.
"""

import sys

for _p in ("/opt/trn_rl_repo", "/root/.axon_site"):
    if _p not in sys.path:
        sys.path.insert(0, _p)

import types
import numpy as np
from contextlib import ExitStack

import concourse.bass as bass
import concourse.tile as tile
from concourse import bacc, mybir
from concourse.bass_utils import run_bass_kernel_spmd
from concourse.vector_clock import ScopedClock


def _lean_epilogue(self, tick_clock, wait_clock):
    """Replaces TileContext._drain_and_barrier for this kernel.

    The stock epilogue costs ~10us on HW: drain + full all-engine barrier
    (~3us of semaphore-propagation latency), per-semaphore clears, then a
    second all-engine barrier. Here GpSimd alone waits for every engine's
    completion clock (same wait set the stock drain used, so the out-DMA
    completion is included), resets DMA state and clears the semaphores for
    re-run correctness; no global barriers. NEFF completion still requires
    all sequencers idle, which orders run N's clears before run N+1.
    """
    nc = self.nc
    drain_inst = nc.gpsimd.drain()
    wait_clock.add_sem_waits(
        drain_inst.ins, ScopedClock({None: tick_clock.global_clock}))
    popped = nc._tile_sem_poison_stack.pop()
    assert popped is self._sem_poison
    nc.clear_and_free_semaphores(list(self.sems.allocated().values()))

H, W = 256, 320
BS = 4
KK = 49
HALF_ROWS = 128
PAD_W = W + 6  # 326
APLANE = 49  # |k| plane j lives at kall plane j + 49

def _ring_ids() -> np.ndarray:
    ring = np.zeros(KK, dtype=np.int32)
    for j in range(KK):
        dy, dx = divmod(j, 7)
        ring[j] = max(abs(dy - 3), abs(dx - 3))
    return ring

_RING = _ring_ids()
RING_TAPS = [np.where(_RING == r)[0].tolist() for r in range(4)]  # 1,8,16,24
RING_ORDER = np.concatenate([np.asarray(t) for t in RING_TAPS]).astype(np.int64)

# plane ranges of each ring inside the [128, 49, 320] ring-ordered k region
RING_OFF = [0, 1, 9, 25, 49]

def _mul_ops(r):
    """Tap-multiply op shapes for ring r>=1: (rel_plane, n_planes, img_dims, img_off).

    Ring taps in j-order: top row (2r+1), middle 2r-1 rows with dx in
    {3-r, 3+r}, bottom row (2r+1). img_dims are AP dims [stride, num]
    prepended to [1, W]; img_off indexes the [7, 326] shifted-image block.
    """
    n = 2 * r + 1
    lo = 3 - r
    return [
        (0, n, [[1, n]], lo * PAD_W + lo),
        (n, 2 * (n - 2), [[PAD_W, n - 2], [2 * r, 2]], (lo + 1) * PAD_W + lo),
        (n + 2 * (n - 2), n, [[1, n]], (lo + n - 1) * PAD_W + lo),
    ]

_NC = None
LAST_RESULTS = None


def _build_program():
    f16 = mybir.dt.float16
    f32 = mybir.dt.float32
    nc = bacc.Bacc("TRN2", target_bir_lowering=False, debug=False, num_devices=8)
    k_d = nc.dram_tensor("k", [HALF_ROWS, KK, W], f16, kind="ExternalInput").ap()
    img7_d = nc.dram_tensor("img7", [HALF_ROWS, 7, PAD_W], f16, kind="ExternalInput").ap()
    in0_d = nc.dram_tensor("in0", [HALF_ROWS, W], f16, kind="ExternalInput").ap()
    att_d = nc.dram_tensor("att", [HALF_ROWS, 4, W], f16, kind="ExternalInput").ap()
    out_d = nc.dram_tensor("out", [HALF_ROWS, W], f16, kind="ExternalOutput").ap()

    tc_obj = tile.TileContext(nc)
    tc_obj._drain_and_barrier = types.MethodType(_lean_epilogue, tc_obj)
    with tc_obj as tc, ExitStack() as ctx:
        pool = ctx.enter_context(tc.tile_pool(name="main", bufs=1))

        k16 = pool.tile([HALF_ROWS, KK, W], f16, name="k16")        # raw k
        # planes 0:49 = patches*k (ring-ordered), planes 49:98 = |k|
        kall = pool.tile([HALF_ROWS, 98, W], f16, name="kall")
        img7_t = pool.tile([HALF_ROWS, 7, PAD_W], f16)
        in0_t = pool.tile([HALF_ROWS, W], f16)
        att_t = pool.tile([HALF_ROWS, 4, W], f16)
        # planes 0:4 = B_r, planes 4:8 = A_r
        res = pool.tile([HALF_ROWS, 8, W], f16)
        pnd = pool.tile([HALF_ROWS, 8, W], f16)

        kall_ap = kall[:]
        kpart = kall_ap.ap[0]
        img7_ap = img7_t[:]
        ipart = img7_ap.ap[0]
        att_ap = att_t[:]

        def kap(plane, dims):
            return bass.AP(kall_ap.tensor, kall_ap.offset + plane * W,
                           [kpart] + dims)

        def iap(off, dims):
            return bass.AP(img7_ap.tensor, img7_ap.offset + off,
                           [ipart] + dims + [[1, W]])

        # ---- DMAs, ordered by when compute needs the data; the first k
        # chunks lead so ScalarE/DVE start ASAP. All transfers cover exact
        # contiguous plane ranges (a strided multi-plane transfer makes the
        # dependency tracker alias its whole bounding box, serializing
        # unrelated compute against the last image rows).
        nc.sync.dma_start(k16[:, 0:5, :], k_d[:, 0:5, :])           # center+r1 top
        nc.sync.dma_start(img7_t[:, 2:5, :], img7_d[:, 2:5, :])     # rows 2,3,4
        nc.sync.dma_start(in0_t[:], in0_d[:])
        nc.sync.dma_start(k16[:, 5:9, :], k_d[:, 5:9, :])           # ring1 rest
        nc.sync.dma_start(k16[:, 9:25, :], k_d[:, 9:25, :])         # ring2
        nc.sync.dma_start(att_t[:], att_d[:])
        nc.sync.dma_start(img7_t[:, 1:2, :], img7_d[:, 1:2, :])
        nc.sync.dma_start(img7_t[:, 5:6, :], img7_d[:, 5:6, :])
        nc.sync.dma_start(k16[:, 25:33, :], k_d[:, 25:33, :])       # ring3
        nc.sync.dma_start(img7_t[:, 0:1, :], img7_d[:, 0:1, :])
        nc.sync.dma_start(img7_t[:, 6:7, :], img7_d[:, 6:7, :])
        nc.sync.dma_start(k16[:, 33:41, :], k_d[:, 33:41, :])
        nc.sync.dma_start(k16[:, 41:49, :], k_d[:, 41:49, :])

        # ---- |k| on ScalarE (reads k16, writes kall[49:98] / res[4]),
        # chunked to follow the k DMA arrivals
        Abs = mybir.ActivationFunctionType.Abs
        nc.scalar.activation(kall[:, 50:54, :], k16[:, 1:5, :], Abs)  # r1 top
        nc.scalar.activation(res[:, 4, :], k16[:, 0, :], Abs)         # A_0
        nc.scalar.activation(kall[:, 54:58, :], k16[:, 5:9, :], Abs)  # r1 rest
        nc.scalar.activation(kall[:, 58:74, :], k16[:, 9:25, :], Abs) # ring2
        nc.scalar.activation(kall[:, 74:90, :], k16[:, 25:41, :], Abs)# ring3a
        nc.scalar.activation(kall[:, 90:98, :], k16[:, 41:49, :], Abs)# ring3b

        # ---- ring sums: each op handles the B level and the A level (49
        # planes up) through a paired leading AP dim
        def paired_fold(base, h, delta):
            """kall[{base, base+49}][0:h] += kall[{base+delta, ...}][0:h]"""
            dims = [[APLANE * W, 2], [W, h], [1, W]]
            nc.vector.tensor_add(kap(base, dims), kap(base, dims),
                                 kap(base + delta, dims))

        def paired_tree(base, sz, r):
            cur = sz
            while cur > 2:
                paired_fold(base, cur // 2, cur // 2)
                cur //= 2
            dims = [[APLANE * W, 2], [1, W]]
            rdims = [[4 * W, 2], [1, W]]
            nc.vector.tensor_add(
                bass.AP(res[:].tensor, res[:].offset + r * W, [res[:].ap[0]] + rdims),
                kap(base, dims), kap(base + 1, dims))

        def ring_muls(r):
            for (rel, n_pl, img_dims, img_off) in _mul_ops(r):
                o = RING_OFF[r] + rel
                nc.vector.tensor_mul(kall[:, o:o + n_pl, :],
                                     k16[:, o:o + n_pl, :],
                                     iap(img_off, img_dims))

        pnd_ap = pnd[:]
        ppart = pnd_ap.ap[0]
        d1 = [[4 * W, 2], [1, W]]

        def pap(plane, dims):
            return bass.AP(pnd_ap.tensor, pnd_ap.offset + plane * W,
                           [ppart] + dims)

        def pnd_mul(r, n):
            """pnd{r..r+n, 4+r..} = res{...} * att{r..r+n} (B|A paired)."""
            dims = [[4 * W, 2], [W, n], [1, W]] if n > 1 else d1
            att_b = bass.AP(att_ap.tensor, att_ap.offset + r * W,
                            [att_ap.ap[0]] + ([[0, 2], [W, n], [1, W]]
                                              if n > 1 else [[0, 2], [1, W]]))
            rsrc = bass.AP(res[:].tensor, res[:].offset + r * W,
                           [res[:].ap[0]] + dims)
            nc.vector.tensor_mul(pap(r, dims), rsrc, att_b)

        # ---- DVE stream, ordered by operand readiness (in-order engine:
        # an op that waits blocks everything issued after it)
        nc.vector.tensor_mul(res[:, 0, :], k16[:, 0, :], in0_t[:])     # B_0
        ring_muls(1)
        paired_tree(1, 8, 1)                     # ring1
        pnd_mul(0, 2)                            # pnd{0,1,4,5} = res*att
        nc.vector.tensor_add(pap(0, d1), pap(0, d1), pap(1, d1))
        ring_muls(2)
        ring_muls(3)
        paired_tree(9, 16, 2)                    # ring2
        pnd_mul(2, 1)
        paired_fold(25, 8, 8)                    # ring3: fold chunks
        paired_fold(25, 8, 16)
        paired_tree(25, 8, 3)
        pnd_mul(3, 1)
        nc.vector.tensor_add(pap(2, d1), pap(2, d1), pap(3, d1))

        nd32 = pool.tile([HALF_ROWS, 2, W], f32)
        nd_dims = [[W, 2], [1, W]]
        nd_ap = bass.AP(nd32[:].tensor, nd32[:].offset, [nd32[:].ap[0]] + nd_dims)
        nc.vector.tensor_add(nd_ap, pap(0, d1), pap(2, d1))  # fp32 N, D

        rden_t = pool.tile([HALF_ROWS, W], f32)
        scr_t = pool.tile([HALF_ROWS, W], f32)
        nc.vector.reciprocal_approx_accurate(rden_t[:], nd32[:, 1, :], scr_t[:])
        out_t = pool.tile([HALF_ROWS, W], f16)
        nc.vector.tensor_mul(out_t[:], nd32[:, 0, :], rden_t[:])
        nc.sync.dma_start(out_d[:], out_t[:])

    nc.compile()
    return nc


def _get_program():
    global _NC
    if _NC is None:
        _NC = _build_program()
    return _NC


def kernel(**inputs) -> np.ndarray:
    k = np.asarray(inputs["kernel"], dtype=np.float32)      # [4, 49, 81920]
    img = np.asarray(inputs["input"], dtype=np.float32)     # [4, 1, 256, 320]
    in0 = np.asarray(inputs["input0"], dtype=np.float32)    # [4, 1, 256, 320]
    att = np.asarray(inputs["attention"], dtype=np.float32) # [4, 6, 4, 81920]
    ii = int(np.asarray(inputs["i"]))

    nc = _get_program()

    in_maps = []
    for c in range(8):
        b, half = divmod(c, 2)
        y0 = half * HALF_ROWS
        kc = k[b][RING_ORDER][:, y0 * W:(y0 + HALF_ROWS) * W]
        kc = np.ascontiguousarray(
            kc.reshape(KK, HALF_ROWS, W).transpose(1, 0, 2).astype(np.float16))
        pad = np.zeros((HALF_ROWS + 6, PAD_W), np.float16)
        lo, hi = max(0, y0 - 3), min(H, y0 + HALF_ROWS + 3)
        pad[lo - (y0 - 3):hi - (y0 - 3), 3:3 + W] = img[b, 0, lo:hi]
        img7 = np.ascontiguousarray(
            np.stack([pad[t:t + HALF_ROWS] for t in range(7)], axis=1))
        in0c = np.ascontiguousarray(in0[b, 0, y0:y0 + HALF_ROWS]).astype(np.float16)
        attc = att[b, ii][:, y0 * W:(y0 + HALF_ROWS) * W]
        attc = np.ascontiguousarray(
            attc.reshape(4, HALF_ROWS, W).transpose(1, 0, 2).astype(np.float16))
        in_maps.append({"k": kc, "img7": img7, "in0": in0c, "att": attc})

    res = run_bass_kernel_spmd(nc, in_maps, list(range(8)))
    global LAST_RESULTS
    LAST_RESULTS = res

    out = np.empty((BS, 1, H, W), np.float32)
    for c in range(8):
        b, half = divmod(c, 2)
        out[b, 0, half * HALF_ROWS:(half + 1) * HALF_ROWS] = \
            res.results[c]["out"].astype(np.float32)
    return out
